# revision 7
# baseline (speedup 1.0000x reference)
"""HMM forward-algorithm log-likelihood on 8 Trainium2 NeuronCores.

Strategy (data-parallel over batch, 8 sequences per core):
  - Work in probability space (scaled forward algorithm): the per-step
    logsumexp over previous states becomes a real matmul v = TT^T-contract,
    done on the PE array with the state vector in (state-partition, batch-free)
    layout so no per-step transposes are needed.
  - Emission log-probs are gathered from a host-transposed bf16 table with
    dma_gather(transpose=True), which lands them directly in
    (state-on-partition, token-on-free) layout, then exp'd on ScalarE with a
    per-partition -logZ bias (the emission log-softmax denominator, computed
    on device in fp32).
  - Per step: 16 bf16 matmuls (4 j-chunks x 4 k-chunks) -> psum v (128,4,8);
    one DVE multiply w = ee * v; 4 matmuls against a ones-vector produce the
    per-sequence state-sum sigma as a (1,8) psum row.  Every 4th step the
    state is renormalized by 1/sigma (broadcast via a rank-1 matmul).
  - All logs are deferred: sigma history (1, 8*1024) is logged once at the
    end, and the answer is a single masked reduction
      L[b] = log sig[idx_b] + sum_{renorm tau < idx_b} log sig[tau].
"""
import numpy as np
import ml_dtypes

import concourse.bass as bass
import concourse.bacc as bacc
import concourse.tile as tile
from concourse import mybir
from concourse import bass_utils

BF16 = ml_dtypes.bfloat16
N = 512
V = 10000
B = 64
TMAX = 1024
NCORES = 8
BS = B // NCORES          # 8 sequences per core
NCH = TMAX // 16          # 64 gather chunks of 128 tokens (16 steps x 8 seqs)
RENORM = 4                # renormalize every 4 steps

_cache = {}


def _build():
    f32 = mybir.dt.float32
    bf16 = mybir.dt.bfloat16
    i16 = mybir.dt.int16
    i32 = mybir.dt.int32
    Exp = mybir.ActivationFunctionType.Exp
    Ln = mybir.ActivationFunctionType.Ln
    Copy = mybir.ActivationFunctionType.Copy
    Alu = mybir.AluOpType

    nc = bacc.Bacc("TRN2")

    transT_d = nc.dram_tensor("transT", (N, N), f32, kind="ExternalInput")
    emit_d = nc.dram_tensor("emit", (N, V), f32, kind="ExternalInput")
    emitT_d = nc.dram_tensor("emitT", (V, N), bf16, kind="ExternalInput")
    priorsT_d = nc.dram_tensor("priorsT", (128, 4), f32, kind="ExternalInput")
    priorsR_d = nc.dram_tensor("priorsR", (1, N), f32, kind="ExternalInput")
    xs_d = nc.dram_tensor("xs", (128, NCH * 8), i16, kind="ExternalInput")
    idxf_d = nc.dram_tensor("idxf", (BS, 1), f32, kind="ExternalInput")
    rmask_d = nc.dram_tensor("rmask", (BS, TMAX), f32, kind="ExternalInput")
    out_d = nc.dram_tensor("out_logp", (BS, 1), f32, kind="ExternalOutput")

    def b3(ap, reps, pos):
        """Insert a stride-0 dim of size `reps` at free position `pos` (1-based in ap list)."""
        newap = list(ap.ap)
        newap.insert(pos, [0, reps])
        return bass.AP(tensor=ap.tensor, offset=ap.offset, ap=newap)

    from contextlib import ExitStack
    with tile.TileContext(nc) as tc, ExitStack() as ctx:
        singles = ctx.enter_context(tc.tile_pool(name="singles", bufs=1))
        bigpool = ctx.enter_context(tc.tile_pool(name="big", bufs=2))
        emitpool = ctx.enter_context(tc.tile_pool(name="emitp", bufs=1))
        epool = ctx.enter_context(tc.tile_pool(name="egather", bufs=4))
        eepool = ctx.enter_context(tc.tile_pool(name="ee", bufs=4))
        wpool = ctx.enter_context(tc.tile_pool(name="w", bufs=3))
        wrpool = ctx.enter_context(tc.tile_pool(name="wrn", bufs=2))
        smallp = ctx.enter_context(tc.tile_pool(name="small", bufs=2))
        vpsum = ctx.enter_context(tc.tile_pool(name="vpsum", bufs=2, space="PSUM"))
        spsum = ctx.enter_context(tc.tile_pool(name="spsum", bufs=2, space="PSUM"))
        bcpsum = ctx.enter_context(tc.tile_pool(name="bcpsum", bufs=2, space="PSUM"))

        # ---------------- constants ----------------
        ones_bf = singles.tile([128, 1], bf16)
        nc.vector.memset(ones_bf[:], 1.0)
        ones_row_f32 = singles.tile([1, 128], f32)
        nc.vector.memset(ones_row_f32[:], 1.0)

        xs_sb = singles.tile([128, NCH * 8], i16)
        nc.sync.dma_start(out=xs_sb[:], in_=xs_d[:])

        # ---------------- TT = softmax(transT rows), bf16 ----------------
        TT = []
        for kc in range(4):
            raw = bigpool.tile([128, N], f32, tag="ttraw")
            nc.sync.dma_start(out=raw[:], in_=transT_d[kc * 128:(kc + 1) * 128, :])
            mx = smallp.tile([128, 1], f32, tag="ttmax")
            nc.vector.tensor_reduce(out=mx[:], in_=raw[:], axis=mybir.AxisListType.X,
                                    op=Alu.max)
            nmx = smallp.tile([128, 1], f32, tag="ttnmax")
            nc.vector.tensor_scalar_mul(nmx[:], mx[:], -1.0)
            ex = bigpool.tile([128, N], f32, tag="ttexp")
            s = smallp.tile([128, 1], f32, tag="ttsum")
            nc.scalar.activation(out=ex[:], in_=raw[:], func=Exp, bias=nmx[:],
                                 scale=1.0, accum_out=s[:])
            rs = smallp.tile([128, 1], f32, tag="ttrs")
            nc.vector.reciprocal(out=rs[:], in_=s[:])
            tt = singles.tile([128, N], bf16, tag=f"tt{kc}")
            nc.scalar.activation(out=tt[:], in_=ex[:], func=Copy, scale=rs[:])
            TT.append(tt)

        # ---------------- logZ per state (fp32), as (128,1) x 4 ----------------
        neglogZ = []
        for kc in range(4):
            erow = emitpool.tile([128, V], f32, tag="emitrow")
            nc.sync.dma_start(out=erow[:], in_=emit_d[kc * 128:(kc + 1) * 128, :])
            mx = smallp.tile([128, 1], f32, tag="lzmax")
            nc.vector.tensor_reduce(out=mx[:], in_=erow[:], axis=mybir.AxisListType.X,
                                    op=Alu.max)
            nmx = smallp.tile([128, 1], f32, tag="lznmax")
            nc.vector.tensor_scalar_mul(nmx[:], mx[:], -1.0)
            junk = emitpool.tile([128, V], bf16, tag="lzjunk")
            s = smallp.tile([128, 1], f32, tag="lzsum")
            nc.scalar.activation(out=junk[:], in_=erow[:], func=Exp, bias=nmx[:],
                                 scale=1.0, accum_out=s[:])
            ls = smallp.tile([128, 1], f32, tag="lzls")
            nc.scalar.activation(out=ls[:], in_=s[:], func=Ln)
            nlz = singles.tile([128, 1], f32, tag=f"nlz{kc}")
            # neglogZ = -(ls + mx)
            nc.vector.tensor_tensor(out=nlz[:], in0=ls[:], in1=mx[:], op=Alu.add)
            nc.vector.tensor_scalar_mul(nlz[:], nlz[:], -1.0)
            neglogZ.append(nlz)

        # ---------------- log-pi bias for t=0 ----------------
        prow = singles.tile([1, N], f32)
        nc.sync.dma_start(out=prow[:], in_=priorsR_d[:])
        pmx = singles.tile([1, 1], f32)
        nc.vector.tensor_reduce(out=pmx[:], in_=prow[:], axis=mybir.AxisListType.X,
                                op=Alu.max)
        npmx = singles.tile([1, 1], f32)
        nc.vector.tensor_scalar_mul(npmx[:], pmx[:], -1.0)
        pjunk = singles.tile([1, N], f32)
        psum_ = singles.tile([1, 1], f32)
        nc.scalar.activation(out=pjunk[:], in_=prow[:], func=Exp, bias=npmx[:],
                             scale=1.0, accum_out=psum_[:])
        pls = singles.tile([1, 1], f32)
        nc.scalar.activation(out=pls[:], in_=psum_[:], func=Ln)
        lse1 = singles.tile([1, 1], f32)
        nc.vector.tensor_tensor(out=lse1[:], in0=pls[:], in1=pmx[:], op=Alu.add)
        lse128 = singles.tile([128, 1], f32)
        nc.gpsimd.partition_broadcast(lse128[:], lse1[:])

        ptr = singles.tile([128, 4], f32)
        nc.sync.dma_start(out=ptr[:], in_=priorsT_d[:])
        pibias = []
        for jc in range(4):
            pb = singles.tile([128, 1], f32, tag=f"pib{jc}")
            # pibias = priorsT[:,jc] - LSE + neglogZ[jc]
            nc.vector.tensor_tensor(out=pb[:], in0=ptr[:, jc:jc + 1], in1=lse128[:],
                                    op=Alu.subtract)
            nc.vector.tensor_tensor(out=pb[:], in0=pb[:], in1=neglogZ[jc][:],
                                    op=Alu.add)
            pibias.append(pb)

        # ---------------- sigma history ----------------
        sighist = singles.tile([BS, TMAX], f32)

        # ---------------- the scan ----------------
        cur_w = None
        for ch in range(NCH):
            eg = epool.tile([128, 4, 128], bf16, tag="eg")
            nc.gpsimd.dma_gather(
                out_ap=eg[:],
                in_ap=emitT_d[:],
                idxs_ap=xs_sb[:, ch * 8:(ch + 1) * 8],
                num_idxs=128,
                num_idxs_reg=128,
                elem_size=N,
                transpose=True,
            )
            ee = eepool.tile([128, 4, 128], bf16, tag="ee")
            for jc in range(4):
                nc.scalar.activation(out=ee[:, jc, :], in_=eg[:, jc, :], func=Exp,
                                     bias=neglogZ[jc][:], scale=1.0)

            sig = spsum.tile([BS, 16], f32, tag="sig")

            for slot in range(16):
                t = ch * 16 + slot
                w = wpool.tile([128, 4, BS], bf16, tag="wt")
                if t == 0:
                    for jc in range(4):
                        nc.scalar.activation(out=w[:, jc, :],
                                             in_=eg[:, jc, 0:BS],
                                             func=Exp, bias=pibias[jc][:], scale=1.0)
                else:
                    v = vpsum.tile([128, 4, BS], f32, tag="v")
                    for jc in range(4):
                        for kc in range(4):
                            nc.tensor.matmul(
                                out=v[:, jc, :],
                                lhsT=TT[kc][:, jc * 128:(jc + 1) * 128],
                                rhs=cur_w[:, kc, :],
                                start=(kc == 0), stop=(kc == 3),
                            )
                    nc.vector.tensor_tensor(
                        out=w[:], in0=v[:],
                        in1=ee[:, :, slot * BS:(slot + 1) * BS], op=Alu.mult)

                sslice = sig[:, slot:slot + 1]
                for jc in range(4):
                    nc.tensor.matmul(out=sslice, lhsT=w[:, jc, :], rhs=ones_bf[:],
                                     start=(jc == 0), stop=(jc == 3))

                if t % RENORM == RENORM - 1:
                    sigrow = spsum.tile([1, BS], f32, tag="sigrow")
                    for jc in range(4):
                        nc.tensor.matmul(out=sigrow[:], lhsT=ones_bf[:],
                                         rhs=w[:, jc, :],
                                         start=(jc == 0), stop=(jc == 3))
                    rinv = smallp.tile([1, BS], f32, tag="rinv")
                    nc.vector.reciprocal(out=rinv[:], in_=sigrow[:])
                    bc = bcpsum.tile([128, BS], f32, tag="bc")
                    nc.tensor.matmul(out=bc[:], lhsT=ones_row_f32[:], rhs=rinv[:],
                                     start=True, stop=True)
                    wr = wrpool.tile([128, 4, BS], bf16, tag="wrn")
                    nc.vector.tensor_tensor(out=wr[:], in0=w[:],
                                            in1=b3(bc[:], 4, 1), op=Alu.mult)
                    cur_w = wr
                else:
                    cur_w = w

            nc.vector.tensor_copy(out=sighist[:, ch * 16:(ch + 1) * 16], in_=sig[:])

        # ---------------- final masked reduction ----------------
        logsig = singles.tile([BS, TMAX], f32)
        nc.scalar.activation(out=logsig[:], in_=sighist[:], func=Ln)

        iota_i = singles.tile([BS, TMAX], i32)
        nc.gpsimd.iota(iota_i[:], pattern=[[1, TMAX]], base=0,
                       channel_multiplier=0)
        iota_f = singles.tile([BS, TMAX], f32)
        nc.vector.tensor_copy(out=iota_f[:], in_=iota_i[:])

        idxf_sb = singles.tile([BS, 1], f32)
        nc.sync.dma_start(out=idxf_sb[:], in_=idxf_d[:])
        rmask_sb = singles.tile([BS, TMAX], f32)
        nc.sync.dma_start(out=rmask_sb[:], in_=rmask_d[:])

        idx_b = b3(idxf_sb[:], TMAX, 1)          # (8, TMAX) free-stride-0
        eq = singles.tile([BS, TMAX], f32)
        nc.vector.tensor_tensor(out=eq[:], in0=iota_f[:], in1=idx_b, op=Alu.is_equal)
        lt = singles.tile([BS, TMAX], f32)
        nc.vector.tensor_tensor(out=lt[:], in0=iota_f[:], in1=idx_b, op=Alu.is_lt)

        mask = singles.tile([BS, TMAX], f32)
        nc.vector.tensor_tensor(out=mask[:], in0=lt[:], in1=rmask_sb[:], op=Alu.mult)
        nc.vector.tensor_tensor(out=mask[:], in0=mask[:], in1=eq[:], op=Alu.add)

        prod = singles.tile([BS, TMAX], f32)
        nc.vector.tensor_tensor(out=prod[:], in0=logsig[:], in1=mask[:], op=Alu.mult)
        Lrow = singles.tile([BS, 1], f32)
        nc.vector.tensor_reduce(out=Lrow[:], in_=prod[:],
                                axis=mybir.AxisListType.X, op=Alu.add)

        nc.sync.dma_start(out=out_d[:], in_=Lrow[:])

    nc.compile()
    return nc


def _prep_inputs(x, T, priors, trans, emit):
    """Host-side sharding + layout prep (no arithmetic beyond index layout)."""
    transT = np.ascontiguousarray(trans.T.astype(np.float32))
    emit_f = np.ascontiguousarray(emit.astype(np.float32))
    emitT = np.ascontiguousarray(emit.T.astype(BF16))
    priorsT = np.ascontiguousarray(priors.astype(np.float32).reshape(4, 128).T)
    priorsR = priors.astype(np.float32).reshape(1, N)

    iota = np.arange(TMAX)
    rmask = np.zeros((BS, TMAX), np.float32)
    rmask[:, (iota % RENORM) == RENORM - 1] = 1.0

    in_maps = []
    for c in range(NCORES):
        xb = x[c * BS:(c + 1) * BS]                  # (8, 1024)
        # gather position i = t_lo*8 + b lives at idx tile [i%16, chunk*8 + i//16]
        xs16 = np.zeros((16, NCH * 8), np.int16)
        for ch in range(NCH):
            for i in range(128):
                t_lo, b = i // BS, i % BS
                xs16[i % 16, ch * 8 + i // 16] = xb[b, ch * 16 + t_lo]
        xs = np.tile(xs16, (8, 1))                   # replicate to 128 partitions
        idx = (np.clip(T[c * BS:(c + 1) * BS], 1, TMAX) - 1).astype(np.float32)
        in_maps.append({
            "transT": transT,
            "emit": emit_f,
            "emitT": emitT,
            "priorsT": priorsT,
            "priorsR": priorsR,
            "xs": xs,
            "idxf": idx.reshape(BS, 1),
            "rmask": rmask,
        })
    return in_maps


def kernel_with_results(x, T, priors, trans, emit, **runkw):
    if "nc" not in _cache:
        _cache["nc"] = _build()
    nc = _cache["nc"]
    in_maps = _prep_inputs(np.asarray(x), np.asarray(T), np.asarray(priors),
                           np.asarray(trans), np.asarray(emit))
    res = bass_utils.run_bass_kernel_spmd(nc, in_maps, core_ids=list(range(NCORES)),
                                          **runkw)
    outs = [np.asarray(r["out_logp"]).reshape(BS) for r in res.results]
    full = np.concatenate(outs).astype(np.float32).reshape(B, 1)
    return full, res


def kernel(x, T, priors, trans, emit):
    out, _ = kernel_with_results(x, T, priors, trans, emit)
    return out


# revision 8
# speedup vs baseline: 1.0104x; 1.0104x over previous
"""HMM forward-algorithm log-likelihood on 8 Trainium2 NeuronCores.

Strategy (data-parallel over batch, 8 sequences per core):
  - Work in probability space (scaled forward algorithm): the per-step
    logsumexp over previous states becomes a real matmul v = TT^T-contract,
    done on the PE array with the state vector in (state-partition, batch-free)
    layout so no per-step transposes are needed.
  - Emission log-probs are gathered from a host-transposed bf16 table with
    dma_gather(transpose=True), which lands them directly in
    (state-on-partition, token-on-free) layout, then exp'd on ScalarE with a
    per-partition -logZ bias (the emission log-softmax denominator, computed
    on device in fp32).
  - Per step: 16 bf16 matmuls (4 j-chunks x 4 k-chunks) -> psum v (128,4,8);
    one DVE multiply w = ee * v; 4 matmuls against a ones-vector produce the
    per-sequence state-sum sigma as a (1,8) psum row.  Every 4th step the
    state is renormalized by 1/sigma (broadcast via a rank-1 matmul).
  - All logs are deferred: sigma history (1, 8*1024) is logged once at the
    end, and the answer is a single masked reduction
      L[b] = log sig[idx_b] + sum_{renorm tau < idx_b} log sig[tau].
"""
import numpy as np
import ml_dtypes

import concourse.bass as bass
import concourse.bacc as bacc
import concourse.tile as tile
from concourse import mybir
from concourse import bass_utils

BF16 = ml_dtypes.bfloat16
N = 512
V = 10000
B = 64
TMAX = 1024
NCORES = 8
BS = B // NCORES          # 8 sequences per core
NCH = TMAX // 16          # 64 gather chunks of 128 tokens (16 steps x 8 seqs)
RENORM = 4                # renormalize every 4 steps

_cache = {}


def _build():
    f32 = mybir.dt.float32
    bf16 = mybir.dt.bfloat16
    i16 = mybir.dt.int16
    i32 = mybir.dt.int32
    Exp = mybir.ActivationFunctionType.Exp
    Ln = mybir.ActivationFunctionType.Ln
    Copy = mybir.ActivationFunctionType.Copy
    Alu = mybir.AluOpType

    nc = bacc.Bacc("TRN2")

    transT_d = nc.dram_tensor("transT", (N, N), f32, kind="ExternalInput")
    emit_d = nc.dram_tensor("emit", (N, V), f32, kind="ExternalInput")
    emitT_d = nc.dram_tensor("emitT", (V, N), bf16, kind="ExternalInput")
    priorsT_d = nc.dram_tensor("priorsT", (128, 4), f32, kind="ExternalInput")
    priorsR_d = nc.dram_tensor("priorsR", (1, N), f32, kind="ExternalInput")
    xs_d = nc.dram_tensor("xs", (128, NCH * 8), i16, kind="ExternalInput")
    idxf_d = nc.dram_tensor("idxf", (BS, 1), f32, kind="ExternalInput")
    rmask_d = nc.dram_tensor("rmask", (BS, TMAX), f32, kind="ExternalInput")
    out_d = nc.dram_tensor("out_logp", (BS, 1), f32, kind="ExternalOutput")

    def b3(ap, reps, pos):
        """Insert a stride-0 dim of size `reps` at free position `pos` (1-based in ap list)."""
        newap = list(ap.ap)
        newap.insert(pos, [0, reps])
        return bass.AP(tensor=ap.tensor, offset=ap.offset, ap=newap)

    from contextlib import ExitStack
    with tile.TileContext(nc) as tc, ExitStack() as ctx:
        singles = ctx.enter_context(tc.tile_pool(name="singles", bufs=1))
        bigpool = ctx.enter_context(tc.tile_pool(name="big", bufs=2))
        emitpool = ctx.enter_context(tc.tile_pool(name="emitp", bufs=1))
        epool = ctx.enter_context(tc.tile_pool(name="egather", bufs=4))
        eepool = ctx.enter_context(tc.tile_pool(name="ee", bufs=4))
        wpool = ctx.enter_context(tc.tile_pool(name="w", bufs=3))
        wrpool = ctx.enter_context(tc.tile_pool(name="wrn", bufs=2))
        smallp = ctx.enter_context(tc.tile_pool(name="small", bufs=2))
        vpsum = ctx.enter_context(tc.tile_pool(name="vpsum", bufs=2, space="PSUM"))
        spsum = ctx.enter_context(tc.tile_pool(name="spsum", bufs=2, space="PSUM"))
        bcpsum = ctx.enter_context(tc.tile_pool(name="bcpsum", bufs=2, space="PSUM"))

        # ---------------- constants ----------------
        ones_bf = singles.tile([128, 1], bf16)
        nc.vector.memset(ones_bf[:], 1.0)
        ones_row_f32 = singles.tile([1, 128], f32)
        nc.vector.memset(ones_row_f32[:], 1.0)

        xs_sb = singles.tile([128, NCH * 8], i16)
        nc.sync.dma_start(out=xs_sb[:], in_=xs_d[:])

        # ---------------- TT = softmax(transT rows), bf16 ----------------
        TT = []
        for kc in range(4):
            raw = bigpool.tile([128, N], f32, tag="ttraw")
            nc.sync.dma_start(out=raw[:], in_=transT_d[kc * 128:(kc + 1) * 128, :])
            mx = smallp.tile([128, 1], f32, tag="ttmax")
            nc.vector.tensor_reduce(out=mx[:], in_=raw[:], axis=mybir.AxisListType.X,
                                    op=Alu.max)
            nmx = smallp.tile([128, 1], f32, tag="ttnmax")
            nc.vector.tensor_scalar_mul(nmx[:], mx[:], -1.0)
            ex = bigpool.tile([128, N], f32, tag="ttexp")
            s = smallp.tile([128, 1], f32, tag="ttsum")
            nc.scalar.activation(out=ex[:], in_=raw[:], func=Exp, bias=nmx[:],
                                 scale=1.0, accum_out=s[:])
            rs = smallp.tile([128, 1], f32, tag="ttrs")
            nc.vector.reciprocal(out=rs[:], in_=s[:])
            tt = singles.tile([128, N], bf16, tag=f"tt{kc}")
            nc.scalar.activation(out=tt[:], in_=ex[:], func=Copy, scale=rs[:])
            TT.append(tt)

        # ---------------- logZ per state (fp32), as (128,1) x 4 ----------------
        neglogZ = []
        for kc in range(4):
            erow = emitpool.tile([128, V], f32, tag="emitrow")
            nc.sync.dma_start(out=erow[:], in_=emit_d[kc * 128:(kc + 1) * 128, :])
            mx = smallp.tile([128, 1], f32, tag="lzmax")
            nc.vector.tensor_reduce(out=mx[:], in_=erow[:], axis=mybir.AxisListType.X,
                                    op=Alu.max)
            nmx = smallp.tile([128, 1], f32, tag="lznmax")
            nc.vector.tensor_scalar_mul(nmx[:], mx[:], -1.0)
            junk = emitpool.tile([128, V], bf16, tag="lzjunk")
            s = smallp.tile([128, 1], f32, tag="lzsum")
            nc.scalar.activation(out=junk[:], in_=erow[:], func=Exp, bias=nmx[:],
                                 scale=1.0, accum_out=s[:])
            ls = smallp.tile([128, 1], f32, tag="lzls")
            nc.scalar.activation(out=ls[:], in_=s[:], func=Ln)
            nlz = singles.tile([128, 1], f32, tag=f"nlz{kc}")
            # neglogZ = -(ls + mx)
            nc.vector.tensor_tensor(out=nlz[:], in0=ls[:], in1=mx[:], op=Alu.add)
            nc.vector.tensor_scalar_mul(nlz[:], nlz[:], -1.0)
            neglogZ.append(nlz)

        # ---------------- log-pi bias for t=0 ----------------
        prow = singles.tile([1, N], f32)
        nc.sync.dma_start(out=prow[:], in_=priorsR_d[:])
        pmx = singles.tile([1, 1], f32)
        nc.vector.tensor_reduce(out=pmx[:], in_=prow[:], axis=mybir.AxisListType.X,
                                op=Alu.max)
        npmx = singles.tile([1, 1], f32)
        nc.vector.tensor_scalar_mul(npmx[:], pmx[:], -1.0)
        pjunk = singles.tile([1, N], f32)
        psum_ = singles.tile([1, 1], f32)
        nc.scalar.activation(out=pjunk[:], in_=prow[:], func=Exp, bias=npmx[:],
                             scale=1.0, accum_out=psum_[:])
        pls = singles.tile([1, 1], f32)
        nc.scalar.activation(out=pls[:], in_=psum_[:], func=Ln)
        lse1 = singles.tile([1, 1], f32)
        nc.vector.tensor_tensor(out=lse1[:], in0=pls[:], in1=pmx[:], op=Alu.add)
        lse128 = singles.tile([128, 1], f32)
        nc.gpsimd.partition_broadcast(lse128[:], lse1[:])

        ptr = singles.tile([128, 4], f32)
        nc.sync.dma_start(out=ptr[:], in_=priorsT_d[:])
        pibias = []
        for jc in range(4):
            pb = singles.tile([128, 1], f32, tag=f"pib{jc}")
            # pibias = priorsT[:,jc] - LSE + neglogZ[jc]
            nc.vector.tensor_tensor(out=pb[:], in0=ptr[:, jc:jc + 1], in1=lse128[:],
                                    op=Alu.subtract)
            nc.vector.tensor_tensor(out=pb[:], in0=pb[:], in1=neglogZ[jc][:],
                                    op=Alu.add)
            pibias.append(pb)

        # ---------------- sigma history ----------------
        sighist = singles.tile([BS, TMAX], f32)

        # ---------------- the scan ----------------
        cur_w = None
        for ch in range(NCH):
            eg = epool.tile([128, 4, 128], bf16, tag="eg")
            nc.gpsimd.dma_gather(
                out_ap=eg[:],
                in_ap=emitT_d[:],
                idxs_ap=xs_sb[:, ch * 8:(ch + 1) * 8],
                num_idxs=128,
                num_idxs_reg=128,
                elem_size=N,
                transpose=True,
            )
            ee = eepool.tile([128, 4, 128], bf16, tag="ee")
            for jc in range(4):
                nc.scalar.activation(out=ee[:, jc, :], in_=eg[:, jc, :], func=Exp,
                                     bias=neglogZ[jc][:], scale=1.0)

            sig = spsum.tile([BS, 16], f32, tag="sig")

            for slot in range(16):
                t = ch * 16 + slot
                w = wpool.tile([128, 4, BS], bf16, tag="wt")
                if t == 0:
                    for jc in range(4):
                        nc.scalar.activation(out=w[:, jc, :],
                                             in_=eg[:, jc, 0:BS],
                                             func=Exp, bias=pibias[jc][:], scale=1.0)
                else:
                    v = vpsum.tile([128, 4, BS], f32, tag="v")
                    for jc in range(4):
                        for kc in range(4):
                            nc.tensor.matmul(
                                out=v[:, jc, :],
                                lhsT=TT[kc][:, jc * 128:(jc + 1) * 128],
                                rhs=cur_w[:, kc, :],
                                start=(kc == 0), stop=(kc == 3),
                            )
                    nc.vector.tensor_tensor(
                        out=w[:], in0=v[:],
                        in1=ee[:, :, slot * BS:(slot + 1) * BS], op=Alu.mult)

                sslice = sig[:, slot:slot + 1]
                for jc in range(4):
                    nc.tensor.matmul(out=sslice, lhsT=w[:, jc, :], rhs=ones_bf[:],
                                     start=(jc == 0), stop=(jc == 3))

                if t % RENORM == RENORM - 1:
                    sigrow = spsum.tile([1, BS], f32, tag="sigrow")
                    for jc in range(4):
                        nc.tensor.matmul(out=sigrow[:], lhsT=ones_bf[:],
                                         rhs=w[:, jc, :],
                                         start=(jc == 0), stop=(jc == 3))
                    rinv = smallp.tile([1, BS], f32, tag="rinv")
                    nc.vector.reciprocal(out=rinv[:], in_=sigrow[:])
                    bc = bcpsum.tile([128, BS], f32, tag="bc")
                    nc.tensor.matmul(out=bc[:], lhsT=ones_row_f32[:], rhs=rinv[:],
                                     start=True, stop=True)
                    wr = wrpool.tile([128, 4, BS], bf16, tag="wrn")
                    nc.vector.tensor_tensor(out=wr[:], in0=w[:],
                                            in1=b3(bc[:], 4, 1), op=Alu.mult)
                    cur_w = wr
                else:
                    cur_w = w

            nc.vector.tensor_copy(out=sighist[:, ch * 16:(ch + 1) * 16], in_=sig[:])

        # ---------------- final masked reduction ----------------
        logsig = singles.tile([BS, TMAX], f32)
        nc.scalar.activation(out=logsig[:], in_=sighist[:], func=Ln)

        iota_i = singles.tile([BS, TMAX], i32)
        nc.gpsimd.iota(iota_i[:], pattern=[[1, TMAX]], base=0,
                       channel_multiplier=0)
        iota_f = singles.tile([BS, TMAX], f32)
        nc.vector.tensor_copy(out=iota_f[:], in_=iota_i[:])

        idxf_sb = singles.tile([BS, 1], f32)
        nc.sync.dma_start(out=idxf_sb[:], in_=idxf_d[:])
        rmask_sb = singles.tile([BS, TMAX], f32)
        nc.sync.dma_start(out=rmask_sb[:], in_=rmask_d[:])

        idx_b = b3(idxf_sb[:], TMAX, 1)          # (8, TMAX) free-stride-0
        eq = singles.tile([BS, TMAX], f32)
        nc.vector.tensor_tensor(out=eq[:], in0=iota_f[:], in1=idx_b, op=Alu.is_equal)
        lt = singles.tile([BS, TMAX], f32)
        nc.vector.tensor_tensor(out=lt[:], in0=iota_f[:], in1=idx_b, op=Alu.is_lt)

        mask = singles.tile([BS, TMAX], f32)
        nc.vector.tensor_tensor(out=mask[:], in0=lt[:], in1=rmask_sb[:], op=Alu.mult)
        nc.vector.tensor_tensor(out=mask[:], in0=mask[:], in1=eq[:], op=Alu.add)

        prod = singles.tile([BS, TMAX], f32)
        nc.vector.tensor_tensor(out=prod[:], in0=logsig[:], in1=mask[:], op=Alu.mult)
        Lrow = singles.tile([BS, 1], f32)
        nc.vector.tensor_reduce(out=Lrow[:], in_=prod[:],
                                axis=mybir.AxisListType.X, op=Alu.add)

        nc.sync.dma_start(out=out_d[:], in_=Lrow[:])

    nc.compile()
    return nc


def _prep_inputs(x, T, priors, trans, emit):
    """Host-side sharding + layout prep (no arithmetic beyond index layout)."""
    transT = np.ascontiguousarray(trans.T.astype(np.float32))
    emit_f = np.ascontiguousarray(emit.astype(np.float32))
    emitT = np.ascontiguousarray(emit.T.astype(BF16))
    priorsT = np.ascontiguousarray(priors.astype(np.float32).reshape(4, 128).T)
    priorsR = priors.astype(np.float32).reshape(1, N)

    iota = np.arange(TMAX)
    rmask = np.zeros((BS, TMAX), np.float32)
    rmask[:, (iota % RENORM) == RENORM - 1] = 1.0

    in_maps = []
    for c in range(NCORES):
        xb = x[c * BS:(c + 1) * BS]                  # (8, 1024)
        # gather position i = t_lo*8 + b lives at idx tile [i%16, chunk*8 + i//16]
        xs16 = np.zeros((16, NCH * 8), np.int16)
        ii, cc = np.meshgrid(np.arange(128), np.arange(NCH), indexing="ij")
        xs16[ii % 16, cc * 8 + ii // 16] = xb[ii % BS, cc * 16 + ii // BS]
        xs = np.tile(xs16, (8, 1))                   # replicate to 128 partitions
        idx = (np.clip(T[c * BS:(c + 1) * BS], 1, TMAX) - 1).astype(np.float32)
        in_maps.append({
            "transT": transT,
            "emit": emit_f,
            "emitT": emitT,
            "priorsT": priorsT,
            "priorsR": priorsR,
            "xs": xs,
            "idxf": idx.reshape(BS, 1),
            "rmask": rmask,
        })
    return in_maps


def kernel_with_results(x, T, priors, trans, emit, **runkw):
    if "nc" not in _cache:
        _cache["nc"] = _build()
    nc = _cache["nc"]
    in_maps = _prep_inputs(np.asarray(x), np.asarray(T), np.asarray(priors),
                           np.asarray(trans), np.asarray(emit))
    res = bass_utils.run_bass_kernel_spmd(nc, in_maps, core_ids=list(range(NCORES)),
                                          **runkw)
    outs = [np.asarray(r["out_logp"]).reshape(BS) for r in res.results]
    full = np.concatenate(outs).astype(np.float32).reshape(B, 1)
    return full, res


def kernel(x, T, priors, trans, emit):
    out, _ = kernel_with_results(x, T, priors, trans, emit)
    return out
